# revision 35
# baseline (speedup 1.0000x reference)
"""Trainium2 Bass kernel for nn_BaseModel_74302934220896 (TuckER + possibility-codebook).

Contract: kernel(**inputs) takes FULL unsharded inputs (as in reference.setup_inputs())
and returns the full output tuple (tucker_logits [B,N] f32, possibility_score [B,N] f32).

Sharding (8 cores):
  - B (2048) -> 8 x 256 for relation/hr/codebook-gather paths
  - N (20000) -> 8 x 2500 (padded to 2560) for tail features and the [B,N] score matmuls
  - head MLP replicated over full B on every core so BN0 needs no collective
  - ONE AllGather carries the per-core [WmT(bf16); interT(bf16)] shards; BN1 stats
    computed locally from the gathered full-B WmT.

v3 design notes:
  - all matmul inputs bf16; outputs bf16 (host upcasts)
  - MLP biases folded into the PSUM->SBUF evacuation (ACT Relu/Identity with
    per-partition bias AP, or DVE scalar_tensor_tensor) -- no separate bias ops
  - Wm / inter einsums ('bc,bcd->bd'): broadcast tensor_tensor multiply
    (alternating GPSIMD-from-SBUF and DVE-from-PSUM paths) + wide last-axis
    tensor_reduce on DVE with bf16 accumulation
  - ta computed directly in [n, c] layout so topk masking needs no pre-transpose
  - score PSUM evacuated in [128,1024] pair-tiles, round-robin DVE/ACT
"""

import sys

sys.path.insert(0, "/opt/trn_rl_repo")

import numpy as np
import ml_dtypes

import concourse.bass as bass
import concourse.bacc as bacc
import concourse.mybir as mybir
import concourse.tile as tile
from concourse.bass import IndirectOffsetOnAxis
from concourse.bass_utils import run_bass_kernel_spmd
from concourse.masks import make_identity

F32 = mybir.dt.float32
BF16 = mybir.dt.bfloat16
I32 = mybir.dt.int32
AF = mybir.ActivationFunctionType
ALU = mybir.AluOpType
AX = mybir.AxisListType

B, N, E, C, R2 = 2048, 20000, 512, 128, 474
NCORES = 8
BSH = B // NCORES            # 256 b rows per core
NSH = N // NCORES            # 2500 tail rows per core
NPAD = 2560                  # padded to 5 groups of 512
NG = NPAD // 512             # 5 n-groups
NB_FULL = B // 128           # 16 b-tiles over full B
TEMP = 0.5
NEG = -1.0e30

_PROG_CACHE = {}


def build_program():
    nc = bacc.Bacc("TRN2", target_bir_lowering=False, debug=False,
                   num_devices=NCORES)

    # ---------------- DRAM I/O ----------------
    dI = lambda name, shape, dt=BF16: nc.dram_tensor(name, shape, dt, kind="ExternalInput")
    headT = dI("headT", [E, B])                    # full-B head_vector^T (rolled)
    relT = dI("relT", [E, BSH])                    # sharded relation_vector^T
    tailT = dI("tailT", [E, NPAD])                 # sharded+padded tail_vector^T
    relidx = dI("relidx", [BSH, 2], I32)
    cbT = dI("cbT", [2 * R2, C * C // 2])          # tanh-input codebook, rows (d-half, c)
    core_dc = dI("core_dc", [C, C * C])            # core as [e, (d, c)] (c fastest)

    hsw1 = dI("hsw1", [E, E]); rsw1 = dI("rsw1", [E, E])
    tsw1 = dI("tsw1", [E, E]); taw1 = dI("taw1", [E, E])
    hsw2 = dI("hsw2", [E, C]); rsw2 = dI("rsw2", [E, C])
    tsw2 = dI("tsw2", [E, C]); taw2 = dI("taw2", [E, C])
    hrw1 = dI("hrw1", [2 * E, 2 * C])
    hrw2 = dI("hrw2", [2 * C, 2 * C])
    hrw3 = dI("hrw3", [2 * C, C])
    # l1 biases chunked [128, nk] f32; l2 biases [128, 1] f32; tab2 row [1, C]
    hsb1 = dI("hsb1", [128, 4], F32); hsb2 = dI("hsb2", [128, 1], F32)
    rsb1 = dI("rsb1", [128, 4], F32); rsb2 = dI("rsb2", [128, 1], F32)
    tsb1 = dI("tsb1", [128, 4], F32); tsb2 = dI("tsb2", [128, 1], F32)
    tab1 = dI("tab1", [128, 4], F32); tab2 = dI("tab2", [1, C])
    hrb1 = dI("hrb1", [128, 2], F32); hrb2 = dI("hrb2", [128, 2], F32)
    hrb3 = dI("hrb3", [128, 1], F32)
    bn0g = dI("bn0g", [128, 1], F32); bn0b = dI("bn0b", [128, 1], F32)
    bn1g = dI("bn1g", [128, 1], F32); bn1b = dI("bn1b", [128, 1], F32)

    tucker = nc.dram_tensor("tucker", [B, NSH], BF16, kind="ExternalOutput")
    poss = nc.dram_tensor("poss", [B, NSH], BF16, kind="ExternalOutput")

    with tile.TileContext(nc) as tc:
        with (
            tc.tile_pool(name="const", bufs=1) as constp,
            tc.tile_pool(name="w1p", bufs=4) as w1p,         # 4 bufs per family tag
            tc.tile_pool(name="hrw1p", bufs=8) as hrw1p,
            tc.tile_pool(name="w2p", bufs=1) as w2p,
            tc.tile_pool(name="big", bufs=1) as bigp,        # core_dc halves (2 tags)
            tc.tile_pool(name="pcp", bufs=2) as pcp,         # gathered codebook tiles
            tc.tile_pool(name="xt", bufs=8) as xtp,
            tc.tile_pool(name="h1", bufs=2) as h1p,
            tc.tile_pool(name="wsb", bufs=2) as wsbp,        # W 4-chunk sbuf tiles
            tc.tile_pool(name="pers", bufs=1) as pers,
            tc.tile_pool(name="small", bufs=2) as smallp,
            tc.tile_pool(name="stage", bufs=4) as stagep,
            tc.tile_pool(name="psA", bufs=2, space="PSUM") as psA,   # [128,2,512] = 2 banks
            tc.tile_pool(name="psB", bufs=2, space="PSUM") as psB,   # [128,512]   = 1 bank
            tc.tile_pool(name="dram", bufs=1, space="DRAM") as dramp,
        ):
            # psC lives through phases 1-3, then is released so psD (scores
            # pipeline depth-3) can use its banks in phase 4.
            psC = tc.alloc_tile_pool(name="psC", bufs=1, space="PSUM")
            # ---------------- constants ----------------
            ident = constp.tile([128, 128], F32)
            make_identity(nc, ident[:])
            ident_bf = constp.tile([128, 128], BF16, tag="idbf")
            nc.gpsimd.tensor_copy(ident_bf[:], ident[:])
            ones_row = constp.tile([1, 512], BF16, tag="ones")
            nc.gpsimd.memset(ones_row[:], 1.0)
            zeros_t = constp.tile([128, 512], BF16, tag="zeros")
            nc.gpsimd.memset(zeros_t[:], 0.0)

            def load_const(dram_t, shape, tag, dt=F32):
                t = constp.tile(shape, dt, tag=tag)
                nc.sync.dma_start(out=t[:], in_=dram_t[:])
                return t

            # biases/weights are loaded lazily, right before their first user,
            # so the head MLP's inputs lead the DMA queue.
            bias = {}
            _bias_spec = {
                "hsb1": (hsb1, [128, 4], F32), "hsb2": (hsb2, [128, 1], F32),
                "rsb1": (rsb1, [128, 4], F32), "rsb2": (rsb2, [128, 1], F32),
                "tsb1": (tsb1, [128, 4], F32), "tsb2": (tsb2, [128, 1], F32),
                "tab1": (tab1, [128, 4], F32), "tab2": (tab2, [1, C], BF16),
                "hrb1": (hrb1, [128, 2], F32), "hrb2": (hrb2, [128, 2], F32),
                "hrb3": (hrb3, [128, 1], F32)}

            def load_bias(*names):
                for nm in names:
                    dr, shp, dt = _bias_spec[nm]
                    bias[nm] = load_const(dr, shp, "b_" + nm, dt)

            def load_w1(w1_dram, nk, tag, pool=None):
                pool = pool or w1p
                w1_t = []
                for k in range(nk):
                    wt = pool.tile([128, w1_dram.shape[1]], BF16, tag=tag)
                    nc.sync.dma_start(out=wt[:], in_=w1_dram[k * 128:(k + 1) * 128, :])
                    w1_t.append(wt)
                return w1_t

            def load_w2(w, nk, tag):
                t = w2p.tile([128, nk, w.shape[1]], BF16, tag=tag)
                nc.sync.dma_start(
                    out=t[:], in_=w[:].rearrange("(k p) c -> p k c", p=128))
                return t

            HALF = C * C // 2

            # persistent tiles
            tsT_s = pers.tile([128, NPAD], BF16)      # ts^T (+biases)
            tamT_s = pers.tile([128, NPAD], BF16)     # tam^T
            WmT_all = pers.tile([128, B], BF16)       # gathered Wm^T raw
            intT_all = pers.tile([128, B], BF16)      # gathered inter^T
            WmT_nb = pers.tile([128, B], BF16)        # BN1-applied, score lhsT
            hsT_full = pers.tile([128, B], F32)       # hs^T full B (pre-BN)
            rsT_bf = pers.tile([128, BSH], BF16)
            WmT_sh = pers.tile([128, BSH], BF16)
            intT_sh = pers.tile([128, BSH], BF16)

            # ---------------- engine routing ----------------
            evac_i = [0]

            def evac(out_ap, ps_ap, which=None):
                """PSUM->SBUF copy routed round-robin DVE/ACT."""
                if which is None:
                    evac_i[0] += 1
                    which = "v" if evac_i[0] % 2 else "a"
                if which == "v":
                    nc.vector.tensor_copy(out_ap, ps_ap)
                else:
                    nc.scalar.activation(out_ap, ps_ap, AF.Copy)
                return which

            relu_i = [0]

            def relu_bias_evac(out_ap, ps_ap, bias_col):
                """out = relu(ps + bias), [128, nb]; ACT-biased routing (2a:1v)."""
                relu_i[0] += 1
                if relu_i[0] % 3 == 0:
                    nb = ps_ap.shape[-1]
                    nc.vector.scalar_tensor_tensor(
                        out_ap, ps_ap, bias_col, zeros_t[:, 0:nb],
                        op0=ALU.add, op1=ALU.max)
                else:
                    nc.scalar.activation(out_ap, ps_ap, AF.Relu, bias=bias_col)

            def bias_evac(out_ap, ps_ap, bias_col):
                """out = ps + bias (per-partition); ACT-biased routing."""
                evac_i[0] += 1
                if evac_i[0] % 3 == 0:
                    nc.vector.tensor_scalar_add(out_ap, ps_ap, bias_col)
                else:
                    nc.scalar.activation(out_ap, ps_ap, AF.Identity, bias=bias_col)

            def mm(out, lhsT, rhs, start=True, stop=True):
                nc.tensor.matmul(out, lhsT, rhs, start=start, stop=stop)

            # ---------------- MLP helpers ----------------
            def load_xt(xT_dram, col0, nb, nk):
                xt_t = []
                for k in range(nk):
                    xt = xtp.tile([128, nb], BF16, tag=f"xt{nb}")
                    nc.sync.dma_start(
                        out=xt[:], in_=xT_dram[k * 128:(k + 1) * 128, col0:col0 + nb])
                    xt_t.append(xt)
                return xt_t

            def mlp_l1(w1_t, b1c, xt_t, nb, nm=4):
                """h1 [128, nm, nb] bf16 = relu(x @ w1 + b1)^T in m-chunks."""
                h1 = h1p.tile([128, nm, nb], BF16, tag=f"h1_{nm}_{nb}")
                for half in range(nm // 2):
                    ps = psA.tile([128, 2, nb], F32, tag="A")
                    for mi in range(2):
                        m = half * 2 + mi
                        for k in range(len(w1_t)):
                            mm(ps[:, mi, :], w1_t[k][:, m * 128:(m + 1) * 128],
                               xt_t[k][:], start=(k == 0), stop=(k == len(w1_t) - 1))
                        relu_bias_evac(h1[:, half * 2 + mi, :], ps[:, mi, :],
                                       b1c[:, m:m + 1])
                return h1

            def mlp_l2_T(w2_s, b2c, h1, nb, out_ap, nm=4):
                """out [128 c, nb] = (h1 @ w2 + b2)^T."""
                ps2 = psB.tile([128, nb], F32, tag="B")
                for m in range(nm):
                    mm(ps2[:], w2_s[:, m, :], h1[:, m, :], start=(m == 0),
                       stop=(m == nm - 1))
                bias_evac(out_ap, ps2[:], b2c[:, 0:1])

            # ---------- soft top-10 mask ([128,128] f32 AP, may be PSUM) ----------
            def topk_mask_mul(x_ap, out_ap):
                """out = sigmoid((x - thr10)/TEMP) * x   (out bf16)"""
                m8 = smallp.tile([128, 8], F32, tag="m8")
                zap = smallp.tile([128, 128], x_ap.dtype, tag="zap")
                nc.vector.max(out=m8[:], in_=x_ap)
                nc.vector.match_replace(out=zap[:], in_to_replace=m8[:],
                                        in_values=x_ap, imm_value=NEG)
                nc.vector.max(out=m8[:], in_=zap[:])
                thr = smallp.tile([128, 1], F32, tag="thr")
                nc.vector.tensor_scalar_mul(thr[:], m8[:, 1:2], -1.0 / TEMP)
                mask = smallp.tile([128, 128], F32, tag="mask")
                nc.scalar.activation(mask[:], x_ap, AF.Sigmoid,
                                     bias=thr[:, 0:1], scale=1.0 / TEMP)
                nc.vector.tensor_mul(out_ap, mask[:], x_ap)

            # =========== rel MLP (first: feeds the W matmuls) ===========
            load_bias("rsb1", "rsb2")
            xt_r = load_xt(relT, 0, BSH, 4)
            rsw1_t = load_w1(rsw1, 4, "w1rs")
            rsw2_s = load_w2(rsw2, 4, "w2b")
            h1r = mlp_l1(rsw1_t, bias["rsb1"], xt_r, BSH)
            mlp_l2_T(rsw2_s, bias["rsb2"], h1r, BSH, rsT_bf[:])

            # =========== hr MLP (3 layers) ===========
            load_bias("hrb1", "hrb2", "hrb3")
            hrw1_t = load_w1(hrw1, 8, "w1hr", hrw1p)
            hrw2_s = load_w2(hrw2, 2, "w2f")
            hrw3_s = load_w2(hrw3, 2, "w2e")
            hr_x = load_xt(headT, 0, BSH, 4) + xt_r   # concat(head_my, rel)
            hrh1 = h1p.tile([128, 2, BSH], BF16, tag="hrh1")
            ps = psA.tile([128, 2, BSH], F32, tag="A")
            for mi in range(2):
                for k in range(8):
                    mm(ps[:, mi, :], hrw1_t[k][:, mi * 128:(mi + 1) * 128],
                       hr_x[k][:], start=(k == 0), stop=(k == 7))
                relu_bias_evac(hrh1[:, mi, :], ps[:, mi, :], bias["hrb1"][:, mi:mi + 1])
            hrh2 = h1p.tile([128, 2, BSH], BF16, tag="hrh2")
            ps = psA.tile([128, 2, BSH], F32, tag="A")
            for mi in range(2):
                for k in range(2):
                    mm(ps[:, mi, :], hrw2_s[:, k, mi * 128:(mi + 1) * 128],
                       hrh1[:, k, :], start=(k == 0), stop=(k == 1))
                relu_bias_evac(hrh2[:, mi, :], ps[:, mi, :], bias["hrb2"][:, mi:mi + 1])
            hraT = smallp.tile([128, BSH], F32, tag="hraT")
            ps2 = psB.tile([128, BSH], F32, tag="B")
            for k in range(2):
                mm(ps2[:], hrw3_s[:, k, :], hrh2[:, k, :], start=(k == 0),
                   stop=(k == 1))
            bias_evac(hraT[:], ps2[:], bias["hrb3"][:, 0:1])

            # hrm (masked hra) in [b, c] layout, bf16, per t-tile
            hrm_bf = []
            for t in range(2):
                pst = psC.tile([128, 128], F32, tag="C")
                nc.tensor.transpose(pst[:], hraT[:, t * 128:(t + 1) * 128], ident[:])
                hb = smallp.tile([128, 128], BF16, tag="hrmbf")
                topk_mask_mul(pst[:], hb[:])
                hrm_bf.append(hb)

            # =========== inter chains (early: GPS mults + DVE trees) ===========
            Wm_sh = [smallp.tile([128, 128], BF16, tag=f"wmsh{t}",
                                 name=f"Wm_sh{t}") for t in range(2)]
            inter_sh = [smallp.tile([128, 128], BF16, tag=f"ish{t}",
                                    name=f"inter_sh{t}") for t in range(2)]
            idx_t = []
            for t in range(2):
                idx = smallp.tile([128, 2], I32, tag=f"idx{t}")
                nc.sync.dma_start(out=idx[:], in_=relidx[t * 128:(t + 1) * 128, :])
                idx_t.append(idx)

            def inter_tile(t, h):
                pc = pcp.tile([128, HALF], BF16, tag="pc")
                nc.gpsimd.indirect_dma_start(
                    out=pc[:], out_offset=None, in_=cbT[:],
                    in_offset=IndirectOffsetOnAxis(ap=idx_t[t][:, h:h + 1], axis=0))
                nc.scalar.activation(pc[:], pc[:], AF.Tanh)
                pc3 = pc[:].rearrange("p (d c) -> p d c", c=128)
                nc.gpsimd.tensor_tensor(
                    out=pc3, in0=pc3,
                    in1=hrm_bf[t][:, None, :].to_broadcast([128, 64, 128]),
                    op=ALU.mult)
                # tree-reduce over c (bf16 TT adds run 2x; tensor_reduce is 1x)
                w = 64
                while w >= 1:
                    nc.vector.tensor_tensor(
                        out=pc3[:, :, 0:w], in0=pc3[:, :, 0:w],
                        in1=pc3[:, :, w:2 * w], op=ALU.add)
                    w //= 2
                nc.vector.tensor_copy(inter_sh[t][:, h * 64:(h + 1) * 64],
                                      pc3[:, :, 0])

            for t in range(2):
                for h in range(2):
                    inter_tile(t, h)

            # =========== head MLP (full B) ===========
            load_bias("hsb1", "hsb2")
            hsw1_t = load_w1(hsw1, 4, "w1hs")
            hsw2_s = load_w2(hsw2, 4, "w2a")
            for bg in range(B // 512):
                xt_bg = load_xt(headT, bg * 512, 512, 4)
                h1h = mlp_l1(hsw1_t, bias["hsb1"], xt_bg, 512)
                mlp_l2_T(hsw2_s, bias["hsb2"], h1h, 512,
                         hsT_full[:, bg * 512:(bg + 1) * 512])

            # =========== BN0 (local, full B) ===========
            def bn_finalize(stats_ap, g_tile, b_tile):
                mv = smallp.tile([128, 2], F32, tag="mv")
                nc.vector.bn_aggr(mv[:], stats_ap)
                scale = smallp.tile([128, 1], F32, tag="sc")
                shift = smallp.tile([128, 1], F32, tag="sh")
                tmp = smallp.tile([128, 1], F32, tag="tm")
                nc.vector.tensor_scalar_add(tmp[:], mv[:, 1:2], 1e-5)
                nc.scalar.activation(scale[:], tmp[:], AF.Sqrt)
                nc.vector.reciprocal(scale[:], scale[:])
                nc.vector.tensor_mul(scale[:], scale[:], g_tile[:, 0:1])
                nc.vector.tensor_mul(tmp[:], mv[:, 0:1], scale[:])
                nc.vector.tensor_sub(shift[:], b_tile[:, 0:1], tmp[:])
                return scale, shift

            bn0g_s = load_const(bn0g, [128, 1], "c11")
            bn0b_s = load_const(bn0b, [128, 1], "c12")
            st0 = smallp.tile([128, 4, 6], F32, tag="st0")
            for i in range(4):
                nc.vector.bn_stats(st0[:, i, :], hsT_full[:, i * 512:(i + 1) * 512])
            bn0_scale, bn0_shift = bn_finalize(st0[:], bn0g_s, bn0b_s)

            # ha (bn0-applied hs shard) in [b, c] bf16 tiles
            haT_bf = smallp.tile([128, BSH], BF16, tag="haTbf")
            nc.vector.tensor_scalar(haT_bf[:], hsT_full[:, 0:BSH], bn0_scale[:, 0:1],
                                    bn0_shift[:, 0:1], op0=ALU.mult, op1=ALU.add)
            ha_bf = []
            for t in range(2):
                pst = psC.tile([128, 128], BF16, tag="Cb")
                nc.tensor.transpose(pst[:], haT_bf[:, t * 128:(t + 1) * 128],
                                    ident_bf[:])
                hb = smallp.tile([128, 128], BF16, tag="habf")
                evac(hb[:], pst[:])
                ha_bf.append(hb)

            # =========== core + tail weights ===========
            core_h = []
            for h in range(2):
                ct = bigp.tile([128, HALF], BF16, tag=f"core{h}", name=f"core{h}")
                nc.sync.dma_start(out=ct[:], in_=core_dc[:, h * HALF:(h + 1) * HALF])
                core_h.append(ct)
            load_bias("tsb1", "tsb2", "tab1", "tab2")
            tsw1_t = load_w1(tsw1, 4, "w1ts")
            taw1_t = load_w1(taw1, 4, "w1ta")
            tsw2_s = load_w2(tsw2, 4, "w2c")
            taw2_s = load_w2(taw2, 4, "w2d")

            # ---------------- W 4-chunk group (DVE multiply + reduce) ----------------
            def w_group(gi):
                t, q = divmod(gi, 8)          # q = which 4-chunk quarter (d0 = 16q)
                wsb = wsbp.tile([128, 4, 4, 128], BF16, tag="wsb")
                for jj in range(4):
                    j = q * 4 + jj            # chunk index 0..31 (d = 4j..4j+3)
                    hsel, off = divmod(j * 512, HALF)
                    pw = psB.tile([128, 512], F32, tag="B")
                    mm(pw[:], rsT_bf[:, t * 128:(t + 1) * 128],
                       core_h[hsel][:, off:off + 512])
                    # DVE multiply straight from PSUM, write bf16 SBUF
                    nc.vector.tensor_tensor(
                        out=wsb[:, jj, :, :],
                        in0=pw[:].rearrange("p (d c) -> p d c", c=128),
                        in1=ha_bf[t][:, None, :].to_broadcast([128, 4, 128]),
                        op=ALU.mult)
                w4 = wsb[:].rearrange("p j d c -> p (j d) c")
                with nc.allow_low_precision("bf16 Wm accumulate"):
                    nc.vector.tensor_reduce(Wm_sh[t][:, 16 * q:16 * (q + 1)],
                                            w4, axis=AX.X, op=ALU.add)

            # =========== Phase 2: all W groups (DVE chain), then AG ===========
            for gi in range(16):
                w_group(gi)

            # =========== Phase 3: pack shard, AllGather (overlaps all tails) ====
            for t in range(2):
                pst = psC.tile([128, 128], BF16, tag="Cb")
                nc.tensor.transpose(pst[:], Wm_sh[t][:], ident_bf[:])
                evac(WmT_sh[:, t * 128:(t + 1) * 128], pst[:], which="a")
                pst2 = psC.tile([128, 128], BF16, tag="Cb")
                nc.tensor.transpose(pst2[:], inter_sh[t][:], ident_bf[:])
                evac(intT_sh[:, t * 128:(t + 1) * 128], pst2[:], which="a")

            ag2_in = dramp.tile([2, 128, BSH], BF16)
            ag2_out = dramp.tile([NCORES, 2, 128, BSH], BF16, addr_space="Shared")
            nc.sync.dma_start(out=ag2_in[0], in_=WmT_sh[:])
            nc.sync.dma_start(out=ag2_in[1], in_=intT_sh[:])
            nc.gpsimd.collective_compute(
                "AllGather", ALU.bypass,
                replica_groups=[list(range(NCORES))],
                ins=[ag2_in.opt()], outs=[ag2_out.opt()])

            # ---------------- tail MLP groups (topk deferred) ----------------
            ta_raw = []        # (g, nt, tile) staged [n, c] bf16 tiles

            def tail_group(g):
                xt_g = load_xt(tailT, g * 512, 512, 4)
                h1t = mlp_l1(tsw1_t, bias["tsb1"], xt_g, 512)
                mlp_l2_T(tsw2_s, bias["tsb2"], h1t, 512,
                         tsT_s[:, g * 512:(g + 1) * 512])
                h1a = mlp_l1(taw1_t, bias["tab1"], xt_g, 512)
                for nt in range(4):
                    pt = psC.tile([128, 128], F32, tag="C")
                    for m in range(4):
                        mm(pt[:], h1a[:, m, nt * 128:(nt + 1) * 128],
                           taw2_s[:, m, :], start=(m == 0), stop=False)
                    mm(pt[:], ones_row[0:1, 0:128], bias["tab2"][0:1, :],
                       start=False, stop=True)
                    raw = smallp.tile([128, 128], BF16, tag="taraw", bufs=20,
                                      name=f"taraw{g}_{nt}")
                    evac(raw[:], pt[:], which="a")
                    ta_raw.append((g, nt, raw))

            for g in range(NG):
                tail_group(g)

            # deferred topk + transpose into tamT (runs during the AllGather)
            for g, nt, raw in ta_raw:
                tam_nc = smallp.tile([128, 128], BF16, tag="tamnc")
                topk_mask_mul(raw[:], tam_nc[:])
                ptT = psC.tile([128, 128], BF16, tag="Cb")
                nc.tensor.transpose(ptT[:], tam_nc[:], ident_bf[:])
                evac(tamT_s[:, g * 512 + nt * 128:g * 512 + (nt + 1) * 128],
                     ptT[:], which="a")
            psC.release()

            nc.sync.dma_start(
                out=WmT_all[:], in_=ag2_out[:, 0].rearrange("r d b -> d r b"))
            nc.sync.dma_start(
                out=intT_all[:], in_=ag2_out[:, 1].rearrange("r d b -> d r b"))

            # BN1 on gathered WmT (full B)
            bn1g_s = load_const(bn1g, [128, 1], "c13")
            bn1b_s = load_const(bn1b, [128, 1], "c14")
            st1 = smallp.tile([128, 4, 6], F32, tag="st1")
            for i in range(4):
                nc.vector.bn_stats(st1[:, i, :], WmT_all[:, i * 512:(i + 1) * 512])
            bn1_scale, bn1_shift = bn_finalize(st1[:], bn1g_s, bn1b_s)
            nc.vector.tensor_scalar(WmT_nb[:], WmT_all[:], bn1_scale[:, 0:1],
                                    bn1_shift[:, 0:1], op0=ALU.mult, op1=ALU.add)

            # =========== Phase 4: scores (quad ring psA/psA/psD = depth 3) ======
            psD = tc.alloc_tile_pool(name="psD", bufs=1, space="PSUM")
            NLAST = NSH - 4 * 512               # 452 valid cols in group 4
            quad_i = [0]

            def quad_tile():
                quad_i[0] += 1
                pool = psD if quad_i[0] % 3 == 0 else psA
                return pool.tile([128, 2, 512], F32, tag="D" if pool is psD else "A",
                                 name=f"psq{quad_i[0]}")

            for bt in range(NB_FULL):
                r0 = bt * 128
                for lhsT, outd, rhs in ((WmT_nb, tucker, tsT_s),
                                        (intT_all, poss, tamT_s)):
                    for half in range(2):
                        psq = quad_tile()
                        for gg in range(2):
                            g = half * 2 + gg
                            mm(psq[:, gg, :], lhsT[:, r0:r0 + 128],
                               rhs[:, g * 512:(g + 1) * 512])
                        stq = stagep.tile([128, 1024], BF16, tag="stq")
                        eng = evac(stq[:], psq[:].rearrange("p g n -> p (g n)"))
                        (nc.sync if eng == "v" else nc.scalar).dma_start(
                            out=outd[r0:r0 + 128, half * 1024:(half + 1) * 1024],
                            in_=stq[:])
                    ps4 = psB.tile([128, 512], F32, tag="B")
                    mm(ps4[:], lhsT[:, r0:r0 + 128], rhs[:, 4 * 512:5 * 512])
                    st4 = stagep.tile([128, 512], BF16, tag="st4")
                    eng = evac(st4[:], ps4[:])
                    (nc.sync if eng == "v" else nc.scalar).dma_start(
                        out=outd[r0:r0 + 128, 2048:NSH], in_=st4[:, 0:NLAST])
            psD.release()

    nc.finalize()
    return nc


# ---------------------------------------------------------------------------
# host side
# ---------------------------------------------------------------------------

BF = np.dtype(ml_dtypes.bfloat16)


def _to_np(x, dt=np.float32):
    return np.ascontiguousarray(np.asarray(x), dtype=dt)


def prepare_in_maps(inputs):
    head = _to_np(inputs["head_vector"])        # [B, E]
    rel = _to_np(inputs["relation_vector"])     # [B, E]
    ridx = np.ascontiguousarray(np.asarray(inputs["relation_index"]).astype(np.int32))
    tailv = _to_np(inputs["tail_vector"])       # [N, E]
    codebook = _to_np(inputs["codebook"])       # [R2, C, C]
    core = _to_np(inputs["core"])               # [C, C, C]

    cbT_host = np.ascontiguousarray(
        codebook.transpose(0, 2, 1).reshape(2 * R2, C * C // 2)).astype(BF)
    core_dc_host = np.ascontiguousarray(
        core.transpose(0, 2, 1).reshape(C, C * C)).astype(BF)

    headT_full = np.ascontiguousarray(head.T).astype(BF)   # [E, B]
    relT_full = np.ascontiguousarray(rel.T).astype(BF)     # [E, B]
    tailT_full = np.ascontiguousarray(tailv.T).astype(BF)  # [E, N]

    def chunked_bias(k, nk):
        return np.ascontiguousarray(_to_np(inputs[k]).reshape(nk, 128).T)

    col = lambda k: _to_np(inputs[k]).reshape(128, 1)
    wcast = lambda k: _to_np(inputs[k]).astype(BF)
    weights_common = {
        "hsw1": wcast("hsw1"), "hsb1": chunked_bias("hsb1", 4),
        "hsw2": wcast("hsw2"), "hsb2": col("hsb2"),
        "rsw1": wcast("rsw1"), "rsb1": chunked_bias("rsb1", 4),
        "rsw2": wcast("rsw2"), "rsb2": col("rsb2"),
        "tsw1": wcast("tsw1"), "tsb1": chunked_bias("tsb1", 4),
        "tsw2": wcast("tsw2"), "tsb2": col("tsb2"),
        "taw1": wcast("taw1"), "tab1": chunked_bias("tab1", 4),
        "taw2": wcast("taw2"),
        "tab2": _to_np(inputs["tab2"]).reshape(1, C).astype(BF),
        "hrw1": wcast("hrw1"), "hrb1": chunked_bias("hrb1", 2),
        "hrw2": wcast("hrw2"), "hrb2": chunked_bias("hrb2", 2),
        "hrw3": wcast("hrw3"), "hrb3": col("hrb3"),
        "bn0g": col("bn0_g"), "bn0b": col("bn0_b"),
        "bn1g": col("bn1_g"), "bn1b": col("bn1_b"),
        "cbT": cbT_host, "core_dc": core_dc_host,
    }

    in_maps = []
    for k in range(NCORES):
        b0 = k * BSH
        n0 = k * NSH
        # rotate headT so THIS core's 256 b-columns come first; BN0 stats are
        # order-invariant and slices [0:256] are "my" shard on every core.
        headT_k = np.ascontiguousarray(np.roll(headT_full, -b0, axis=1))
        tailT_k = np.zeros((E, NPAD), BF)
        tailT_k[:, :NSH] = tailT_full[:, n0:n0 + NSH]
        m = dict(weights_common)
        m["headT"] = headT_k
        m["relT"] = np.ascontiguousarray(relT_full[:, b0:b0 + BSH])
        m["tailT"] = tailT_k
        ri = ridx[b0:b0 + BSH]
        m["relidx"] = np.ascontiguousarray(
            np.stack([2 * ri, 2 * ri + 1], axis=1))
        in_maps.append(m)
    return in_maps


def assemble_outputs(results):
    tuckers, posses = [], []
    for k in range(NCORES):
        r = results[k]
        tuckers.append(np.asarray(r["tucker"]).astype(np.float32))
        posses.append(np.asarray(r["poss"]).astype(np.float32))
    tucker_full = np.concatenate(tuckers, axis=1)
    poss_full = np.concatenate(posses, axis=1)
    return tucker_full, poss_full


def kernel(**inputs):
    if "prog" not in _PROG_CACHE:
        _PROG_CACHE["prog"] = build_program()
    nc = _PROG_CACHE["prog"]
    in_maps = prepare_in_maps(inputs)
    res = run_bass_kernel_spmd(nc, in_maps, list(range(NCORES)))
    return assemble_outputs(res.results)


# revision 46
# speedup vs baseline: 1.1854x; 1.1854x over previous
"""Trainium2 Bass kernel for nn_BaseModel_74302934220896 (TuckER + possibility-codebook).

Contract: kernel(**inputs) takes FULL unsharded inputs (as in reference.setup_inputs())
and returns the full output tuple (tucker_logits [B,N] f32, possibility_score [B,N] f32).

Sharding (8 cores):
  - B (2048) -> 8 x 256 for relation/hr/codebook-gather paths
  - N (20000) -> 8 x 2500 (padded to 2560) for tail features and the [B,N] score matmuls
  - head MLP replicated over full B on every core so BN0 needs no collective
  - ONE AllGather carries the per-core [WmT(bf16); interT(bf16)] shards; BN1 stats
    computed locally from the gathered full-B WmT.

v3 design notes:
  - all matmul inputs bf16; outputs bf16 (host upcasts)
  - MLP biases folded into the PSUM->SBUF evacuation (ACT Relu/Identity with
    per-partition bias AP, or DVE scalar_tensor_tensor) -- no separate bias ops
  - Wm / inter einsums ('bc,bcd->bd'): broadcast tensor_tensor multiply
    (alternating GPSIMD-from-SBUF and DVE-from-PSUM paths) + wide last-axis
    tensor_reduce on DVE with bf16 accumulation
  - ta computed directly in [n, c] layout so topk masking needs no pre-transpose
  - score PSUM evacuated in [128,1024] pair-tiles, round-robin DVE/ACT
"""

import sys

sys.path.insert(0, "/opt/trn_rl_repo")

import numpy as np
import ml_dtypes

import concourse.bass as bass
import concourse.bacc as bacc
import concourse.mybir as mybir
import concourse.tile as tile
from concourse.bass import IndirectOffsetOnAxis
from concourse.bass_utils import run_bass_kernel_spmd
from concourse.masks import make_identity

F32 = mybir.dt.float32
BF16 = mybir.dt.bfloat16
I32 = mybir.dt.int32
AF = mybir.ActivationFunctionType
ALU = mybir.AluOpType
AX = mybir.AxisListType

B, N, E, C, R2 = 2048, 20000, 512, 128, 474
NCORES = 8
BSH = B // NCORES            # 256 b rows per core
NSH = N // NCORES            # 2500 tail rows per core
NPAD = 2560                  # padded to 5 groups of 512
NG = NPAD // 512             # 5 n-groups
NB_FULL = B // 128           # 16 b-tiles over full B
TEMP = 0.5
NEG = -1.0e30

_PROG_CACHE = {}


def build_program():
    nc = bacc.Bacc("TRN2", target_bir_lowering=False, debug=False,
                   num_devices=NCORES)

    # ---------------- DRAM I/O ----------------
    dI = lambda name, shape, dt=BF16: nc.dram_tensor(name, shape, dt, kind="ExternalInput")
    headT = dI("headT", [E, B])                    # full-B head_vector^T (rolled)
    relT = dI("relT", [E, BSH])                    # sharded relation_vector^T
    tailT = dI("tailT", [E, NPAD])                 # sharded+padded tail_vector^T
    relidx = dI("relidx", [BSH, 2], I32)
    cbT = dI("cbT", [2 * R2, C * C // 2])          # tanh-input codebook, rows (d-half, c)
    core_dc = dI("core_dc", [C, C * C])            # core as [e, (d, c)] (c fastest)

    hsw1 = dI("hsw1", [E, E]); rsw1 = dI("rsw1", [E, E])
    tsw1 = dI("tsw1", [E, E]); taw1 = dI("taw1", [E, E])
    hsw2 = dI("hsw2", [E, C]); rsw2 = dI("rsw2", [E, C])
    tsw2 = dI("tsw2", [E, C]); taw2 = dI("taw2", [E, C])
    hrw1 = dI("hrw1", [2 * E, 2 * C])
    hrw2 = dI("hrw2", [2 * C, 2 * C])
    hrw3 = dI("hrw3", [2 * C, C])
    # l1 biases chunked [128, nk] f32; l2 biases [128, 1] f32; tab2 row [1, C]
    hsb1 = dI("hsb1", [128, 4], F32); hsb2 = dI("hsb2", [128, 1], F32)
    rsb1 = dI("rsb1", [128, 4], F32); rsb2 = dI("rsb2", [128, 1], F32)
    tsb1 = dI("tsb1", [128, 4], F32); tsb2 = dI("tsb2", [128, 1], F32)
    tab1 = dI("tab1", [128, 4], F32); tab2 = dI("tab2", [1, C])
    hrb1 = dI("hrb1", [128, 2], F32); hrb2 = dI("hrb2", [128, 2], F32)
    hrb3 = dI("hrb3", [128, 1], F32)
    bn0g = dI("bn0g", [128, 1], F32); bn0b = dI("bn0b", [128, 1], F32)
    bn1g = dI("bn1g", [128, 1], F32); bn1b = dI("bn1b", [128, 1], F32)

    tucker = nc.dram_tensor("tucker", [B, NSH], BF16, kind="ExternalOutput")
    poss = nc.dram_tensor("poss", [B, NSH], BF16, kind="ExternalOutput")

    with tile.TileContext(nc) as tc:
        with (
            tc.tile_pool(name="const", bufs=1) as constp,
            tc.tile_pool(name="w1p", bufs=4) as w1p,         # 4 bufs per family tag
            tc.tile_pool(name="hrw1p", bufs=8) as hrw1p,
            tc.tile_pool(name="w2p", bufs=1) as w2p,
            tc.tile_pool(name="big", bufs=1) as bigp,        # core_dc halves (2 tags)
            tc.tile_pool(name="pcp", bufs=2) as pcp,         # gathered codebook tiles
            tc.tile_pool(name="xt", bufs=8) as xtp,
            tc.tile_pool(name="h1", bufs=2) as h1p,
            tc.tile_pool(name="wsb", bufs=2) as wsbp,        # W 4-chunk sbuf tiles
            tc.tile_pool(name="pers", bufs=1) as pers,
            tc.tile_pool(name="small", bufs=2) as smallp,
            tc.tile_pool(name="stage", bufs=3) as stagep,
            tc.tile_pool(name="psA", bufs=2, space="PSUM") as psA,   # [128,2,512] = 2 banks
            tc.tile_pool(name="psB", bufs=2, space="PSUM") as psB,   # [128,512]   = 1 bank
            tc.tile_pool(name="dram", bufs=1, space="DRAM") as dramp,
        ):
            # psC lives through phases 1-3, then is released so psD (scores
            # pipeline depth-3) can use its banks in phase 4.
            psC = tc.alloc_tile_pool(name="psC", bufs=1, space="PSUM")
            # ---------------- constants ----------------
            ident = constp.tile([128, 128], F32)
            make_identity(nc, ident[:])
            ident_bf = constp.tile([128, 128], BF16, tag="idbf")
            nc.gpsimd.tensor_copy(ident_bf[:], ident[:])
            ones_row = constp.tile([1, 512], BF16, tag="ones")
            nc.gpsimd.memset(ones_row[:], 1.0)
            zeros_t = constp.tile([128, 512], BF16, tag="zeros")
            nc.gpsimd.memset(zeros_t[:], 0.0)

            def load_const(dram_t, shape, tag, dt=F32):
                t = constp.tile(shape, dt, tag=tag)
                nc.sync.dma_start(out=t[:], in_=dram_t[:])
                return t

            # biases/weights are loaded lazily, right before their first user,
            # so the head MLP's inputs lead the DMA queue.
            bias = {}
            _bias_spec = {
                "hsb1": (hsb1, [128, 4], F32), "hsb2": (hsb2, [128, 1], F32),
                "rsb1": (rsb1, [128, 4], F32), "rsb2": (rsb2, [128, 1], F32),
                "tsb1": (tsb1, [128, 4], F32), "tsb2": (tsb2, [128, 1], F32),
                "tab1": (tab1, [128, 4], F32), "tab2": (tab2, [1, C], BF16),
                "hrb1": (hrb1, [128, 2], F32), "hrb2": (hrb2, [128, 2], F32),
                "hrb3": (hrb3, [128, 1], F32)}

            def load_bias(*names):
                for nm in names:
                    dr, shp, dt = _bias_spec[nm]
                    bias[nm] = load_const(dr, shp, "b_" + nm, dt)

            def load_w1(w1_dram, nk, tag, pool=None):
                pool = pool or w1p
                w1_t = []
                for k in range(nk):
                    wt = pool.tile([128, w1_dram.shape[1]], BF16, tag=tag)
                    nc.sync.dma_start(out=wt[:], in_=w1_dram[k * 128:(k + 1) * 128, :])
                    w1_t.append(wt)
                return w1_t

            def load_w2(w, nk, tag):
                t = w2p.tile([128, nk, w.shape[1]], BF16, tag=tag)
                nc.sync.dma_start(
                    out=t[:], in_=w[:].rearrange("(k p) c -> p k c", p=128))
                return t

            HALF = C * C // 2

            # persistent tiles
            tsT_s = pers.tile([128, NPAD], BF16)      # ts^T (+biases)
            tamT_s = pers.tile([128, NPAD], BF16)     # tam^T
            WmT_all = pers.tile([128, B], BF16)       # gathered Wm^T raw
            intT_all = pers.tile([128, B], BF16)      # gathered inter^T
            WmT_nb = pers.tile([128, B], BF16)        # BN1-applied, score lhsT
            hsT_full = pers.tile([128, B], F32)       # hs^T full B (pre-BN)
            rsT_bf = pers.tile([128, BSH], BF16)
            WmT_sh = pers.tile([128, BSH], BF16)
            intT_sh = pers.tile([128, BSH], BF16)

            # ---------------- engine routing ----------------
            evac_i = [0]

            def evac(out_ap, ps_ap, which=None):
                """PSUM->SBUF copy routed round-robin DVE/ACT."""
                if which is None:
                    evac_i[0] += 1
                    which = "v" if evac_i[0] % 2 else "a"
                if which == "v":
                    nc.vector.tensor_copy(out_ap, ps_ap)
                else:
                    nc.scalar.activation(out_ap, ps_ap, AF.Copy)
                return which

            relu_i = [0]

            def relu_bias_evac(out_ap, ps_ap, bias_col):
                """out = relu(ps + bias), [128, nb]; ACT-biased routing (2a:1v)."""
                relu_i[0] += 1
                if relu_i[0] % 3 == 0:
                    nb = ps_ap.shape[-1]
                    nc.vector.scalar_tensor_tensor(
                        out_ap, ps_ap, bias_col, zeros_t[:, 0:nb],
                        op0=ALU.add, op1=ALU.max)
                else:
                    nc.scalar.activation(out_ap, ps_ap, AF.Relu, bias=bias_col)

            def bias_evac(out_ap, ps_ap, bias_col):
                """out = ps + bias (per-partition); ACT-biased routing."""
                evac_i[0] += 1
                if evac_i[0] % 3 == 0:
                    nc.vector.tensor_scalar_add(out_ap, ps_ap, bias_col)
                else:
                    nc.scalar.activation(out_ap, ps_ap, AF.Identity, bias=bias_col)

            def mm(out, lhsT, rhs, start=True, stop=True):
                nc.tensor.matmul(out, lhsT, rhs, start=start, stop=stop)

            # ---------------- MLP helpers ----------------
            def load_xt(xT_dram, col0, nb, nk):
                xt_t = []
                for k in range(nk):
                    xt = xtp.tile([128, nb], BF16, tag=f"xt{nb}",
                                  bufs=8 if nb == BSH else 6)
                    nc.sync.dma_start(
                        out=xt[:], in_=xT_dram[k * 128:(k + 1) * 128, col0:col0 + nb])
                    xt_t.append(xt)
                return xt_t

            def mlp_l1(w1_t, b1c, xt_t, nb, nm=4):
                """h1 [128, nm, nb] bf16 = relu(x @ w1 + b1)^T in m-chunks."""
                h1 = h1p.tile([128, nm, nb], BF16, tag=f"h1_{nm}_{nb}")
                for half in range(nm // 2):
                    ps = psA.tile([128, 2, nb], F32, tag="A")
                    for mi in range(2):
                        m = half * 2 + mi
                        for k in range(len(w1_t)):
                            mm(ps[:, mi, :], w1_t[k][:, m * 128:(m + 1) * 128],
                               xt_t[k][:], start=(k == 0), stop=(k == len(w1_t) - 1))
                        relu_bias_evac(h1[:, half * 2 + mi, :], ps[:, mi, :],
                                       b1c[:, m:m + 1])
                return h1

            def mlp_l2_T(w2_s, b2c, h1, nb, out_ap, nm=4):
                """out [128 c, nb] = (h1 @ w2 + b2)^T."""
                ps2 = psB.tile([128, nb], F32, tag="B")
                for m in range(nm):
                    mm(ps2[:], w2_s[:, m, :], h1[:, m, :], start=(m == 0),
                       stop=(m == nm - 1))
                bias_evac(out_ap, ps2[:], b2c[:, 0:1])

            # ---------- soft top-10 mask ([128,128] f32 AP, may be PSUM) ----------
            def topk_mask_mul(x_ap, out_ap):
                """out = sigmoid((x - thr10)/TEMP) * x   (out bf16)"""
                m8 = smallp.tile([128, 8], F32, tag="m8")
                zap = smallp.tile([128, 128], x_ap.dtype, tag="zap")
                nc.vector.max(out=m8[:], in_=x_ap)
                nc.vector.match_replace(out=zap[:], in_to_replace=m8[:],
                                        in_values=x_ap, imm_value=NEG)
                nc.vector.max(out=m8[:], in_=zap[:])
                thr = smallp.tile([128, 1], F32, tag="thr")
                nc.vector.tensor_scalar_mul(thr[:], m8[:, 1:2], -1.0 / TEMP)
                mask = smallp.tile([128, 128], F32, tag="mask")
                nc.scalar.activation(mask[:], x_ap, AF.Sigmoid,
                                     bias=thr[:, 0:1], scale=1.0 / TEMP)
                nc.vector.tensor_mul(out_ap, mask[:], x_ap)

            # =========== rel MLP (first: feeds the W matmuls) ===========
            load_bias("rsb1", "rsb2")
            xt_r = load_xt(relT, 0, BSH, 4)
            rsw1_t = load_w1(rsw1, 4, "w1rs")
            rsw2_s = load_w2(rsw2, 4, "w2b")
            h1r = mlp_l1(rsw1_t, bias["rsb1"], xt_r, BSH)
            mlp_l2_T(rsw2_s, bias["rsb2"], h1r, BSH, rsT_bf[:])

            # =========== hr MLP (3 layers) ===========
            load_bias("hrb1", "hrb2", "hrb3")
            hrw1_t = load_w1(hrw1, 8, "w1hr", hrw1p)
            hrw2_s = load_w2(hrw2, 2, "w2f")
            hrw3_s = load_w2(hrw3, 2, "w2e")
            hr_x = load_xt(headT, 0, BSH, 4) + xt_r   # concat(head_my, rel)
            hrh1 = h1p.tile([128, 2, BSH], BF16, tag="hrh1", bufs=1)
            ps = psA.tile([128, 2, BSH], F32, tag="A")
            for mi in range(2):
                for k in range(8):
                    mm(ps[:, mi, :], hrw1_t[k][:, mi * 128:(mi + 1) * 128],
                       hr_x[k][:], start=(k == 0), stop=(k == 7))
                relu_bias_evac(hrh1[:, mi, :], ps[:, mi, :], bias["hrb1"][:, mi:mi + 1])
            hrh2 = h1p.tile([128, 2, BSH], BF16, tag="hrh2", bufs=1)
            ps = psA.tile([128, 2, BSH], F32, tag="A")
            for mi in range(2):
                for k in range(2):
                    mm(ps[:, mi, :], hrw2_s[:, k, mi * 128:(mi + 1) * 128],
                       hrh1[:, k, :], start=(k == 0), stop=(k == 1))
                relu_bias_evac(hrh2[:, mi, :], ps[:, mi, :], bias["hrb2"][:, mi:mi + 1])
            hraT = smallp.tile([128, BSH], F32, tag="hraT")
            ps2 = psB.tile([128, BSH], F32, tag="B")
            for k in range(2):
                mm(ps2[:], hrw3_s[:, k, :], hrh2[:, k, :], start=(k == 0),
                   stop=(k == 1))
            bias_evac(hraT[:], ps2[:], bias["hrb3"][:, 0:1])

            # hrm (masked hra) in [b, c] layout, bf16, per t-tile
            hrm_bf = []
            for t in range(2):
                pst = psC.tile([128, 128], F32, tag="C")
                nc.tensor.transpose(pst[:], hraT[:, t * 128:(t + 1) * 128], ident[:])
                hb = smallp.tile([128, 128], BF16, tag="hrmbf")
                topk_mask_mul(pst[:], hb[:])
                hrm_bf.append(hb)

            # =========== inter gathers kicked off early (SWDGE ring) ===========
            Wm_sh = [smallp.tile([128, 128], BF16, tag=f"wmsh{t}",
                                 name=f"Wm_sh{t}") for t in range(2)]
            inter_sh = [smallp.tile([128, 128], BF16, tag=f"ish{t}",
                                    name=f"inter_sh{t}") for t in range(2)]
            idx_t = []
            for t in range(2):
                idx = smallp.tile([128, 2], I32, tag=f"idx{t}")
                nc.sync.dma_start(out=idx[:], in_=relidx[t * 128:(t + 1) * 128, :])
                idx_t.append(idx)

            pc_tiles = {}
            for t in range(2):
                for h in range(2):
                    pc = pcp.tile([128, HALF], BF16, tag=f"pc{h}",
                                  name=f"pc{t}{h}")
                    nc.gpsimd.indirect_dma_start(
                        out=pc[:], out_offset=None, in_=cbT[:],
                        in_offset=IndirectOffsetOnAxis(ap=idx_t[t][:, h:h + 1],
                                                       axis=0))
                    pc_tiles[(t, h)] = pc

            def inter_mult(t, h):
                """tanh (ACT) + hrm broadcast multiply (GPS), in place."""
                pc = pc_tiles[(t, h)]
                nc.scalar.activation(pc[:], pc[:], AF.Tanh)
                pc3 = pc[:].rearrange("p (d c) -> p d c", c=128)
                nc.gpsimd.tensor_tensor(
                    out=pc3, in0=pc3,
                    in1=hrm_bf[t][:, None, :].to_broadcast([128, 64, 128]),
                    op=ALU.mult)

            def inter_tree(t, h):
                """tree-reduce over c on DVE (bf16 TT adds)."""
                pc3 = pc_tiles[(t, h)][:].rearrange("p (d c) -> p d c", c=128)
                w = 64
                while w >= 1:
                    nc.vector.tensor_tensor(
                        out=pc3[:, :, 0:w], in0=pc3[:, :, 0:w],
                        in1=pc3[:, :, w:2 * w], op=ALU.add)
                    w //= 2
                nc.vector.tensor_copy(inter_sh[t][:, h * 64:(h + 1) * 64],
                                      pc3[:, :, 0])

            # =========== head MLP (full B) ===========
            load_bias("hsb1", "hsb2")
            hsw1_t = load_w1(hsw1, 4, "w1hs")
            hsw2_s = load_w2(hsw2, 4, "w2a")
            for bg in range(B // 512):
                xt_bg = load_xt(headT, bg * 512, 512, 4)
                h1h = mlp_l1(hsw1_t, bias["hsb1"], xt_bg, 512)
                mlp_l2_T(hsw2_s, bias["hsb2"], h1h, 512,
                         hsT_full[:, bg * 512:(bg + 1) * 512])

            # =========== BN0 (local, full B) ===========
            def bn_finalize(stats_ap, g_tile, b_tile):
                mv = smallp.tile([128, 2], F32, tag="mv")
                nc.vector.bn_aggr(mv[:], stats_ap)
                scale = smallp.tile([128, 1], F32, tag="sc")
                shift = smallp.tile([128, 1], F32, tag="sh")
                tmp = smallp.tile([128, 1], F32, tag="tm")
                nc.vector.tensor_scalar_add(tmp[:], mv[:, 1:2], 1e-5)
                nc.scalar.activation(scale[:], tmp[:], AF.Sqrt)
                nc.vector.reciprocal(scale[:], scale[:])
                nc.vector.tensor_mul(scale[:], scale[:], g_tile[:, 0:1])
                nc.vector.tensor_mul(tmp[:], mv[:, 0:1], scale[:])
                nc.vector.tensor_sub(shift[:], b_tile[:, 0:1], tmp[:])
                return scale, shift

            bn0g_s = load_const(bn0g, [128, 1], "c11")
            bn0b_s = load_const(bn0b, [128, 1], "c12")
            st0 = smallp.tile([128, 4, 6], F32, tag="st0")
            for i in range(4):
                nc.vector.bn_stats(st0[:, i, :], hsT_full[:, i * 512:(i + 1) * 512])
            bn0_scale, bn0_shift = bn_finalize(st0[:], bn0g_s, bn0b_s)

            # ha (bn0-applied hs shard) in [b, c] bf16 tiles
            haT_bf = smallp.tile([128, BSH], BF16, tag="haTbf")
            nc.vector.tensor_scalar(haT_bf[:], hsT_full[:, 0:BSH], bn0_scale[:, 0:1],
                                    bn0_shift[:, 0:1], op0=ALU.mult, op1=ALU.add)
            ha_bf = []
            for t in range(2):
                pst = psC.tile([128, 128], BF16, tag="Cb")
                nc.tensor.transpose(pst[:], haT_bf[:, t * 128:(t + 1) * 128],
                                    ident_bf[:])
                hb = smallp.tile([128, 128], BF16, tag="habf")
                evac(hb[:], pst[:])
                ha_bf.append(hb)

            # =========== core + tail weights ===========
            core_h = []
            for h in range(2):
                ct = bigp.tile([128, HALF], BF16, tag=f"core{h}", name=f"core{h}")
                nc.sync.dma_start(out=ct[:], in_=core_dc[:, h * HALF:(h + 1) * HALF])
                core_h.append(ct)
            load_bias("tsb1", "tsb2", "tab1", "tab2")
            tsw1_t = load_w1(tsw1, 4, "w1hs")   # reuse head-w1 slots (head done)
            taw1_t = load_w1(taw1, 4, "w1rs")   # reuse rel-w1 slots
            tsw2_s = load_w2(tsw2, 4, "w2c")
            taw2_s = load_w2(taw2, 4, "w2d")

            # ---------------- W 4-chunk group (DVE multiply + reduce) ----------------
            def w_group(gi):
                t, q = divmod(gi, 8)          # q = which 4-chunk quarter (d0 = 16q)
                wsb = wsbp.tile([128, 4, 4, 128], BF16, tag="wsb")
                for jj in range(4):
                    j = q * 4 + jj            # chunk index 0..31 (d = 4j..4j+3)
                    hsel, off = divmod(j * 512, HALF)
                    pw = psB.tile([128, 512], F32, tag="B")
                    mm(pw[:], rsT_bf[:, t * 128:(t + 1) * 128],
                       core_h[hsel][:, off:off + 512])
                    # DVE multiply straight from PSUM, write bf16 SBUF
                    nc.vector.tensor_tensor(
                        out=wsb[:, jj, :, :],
                        in0=pw[:].rearrange("p (d c) -> p d c", c=128),
                        in1=ha_bf[t][:, None, :].to_broadcast([128, 4, 128]),
                        op=ALU.mult)
                w4 = wsb[:].rearrange("p j d c -> p (j d) c")
                with nc.allow_low_precision("bf16 Wm accumulate"):
                    nc.vector.tensor_reduce(Wm_sh[t][:, 16 * q:16 * (q + 1)],
                                            w4, axis=AX.X, op=ALU.add)

            # ---------------- tail MLP group (topk deferred) ----------------
            ta_raw = []        # (g, nt, tile) staged [n, c] bf16 tiles

            def tail_group(g):
                xt_g = load_xt(tailT, g * 512, 512, 4)
                h1t = mlp_l1(tsw1_t, bias["tsb1"], xt_g, 512)
                mlp_l2_T(tsw2_s, bias["tsb2"], h1t, 512,
                         tsT_s[:, g * 512:(g + 1) * 512])
                h1a = mlp_l1(taw1_t, bias["tab1"], xt_g, 512)
                for nt in range(4):
                    pt = psC.tile([128, 128], F32, tag="C")
                    for m in range(4):
                        mm(pt[:], h1a[:, m, nt * 128:(nt + 1) * 128],
                           taw2_s[:, m, :], start=(m == 0), stop=False)
                    mm(pt[:], ones_row[0:1, 0:128], bias["tab2"][0:1, :],
                       start=False, stop=True)
                    raw = smallp.tile([128, 128], BF16, tag="taraw", bufs=20,
                                      name=f"taraw{g}_{nt}")
                    evac(raw[:], pt[:], which="a")
                    ta_raw.append((g, nt, raw))

            # =========== Phase 2: W chain (DVE) + inter mults, tails fill =======
            for blk in range(4):
                for gi in range(blk * 4, blk * 4 + 4):
                    w_group(gi)
                inter_mult(blk // 2, blk % 2)
                if blk < 3:
                    tail_group(blk)

            # =========== Phase 3a: AllGather #1 (Wm) ===========
            for t in range(2):
                pst = psC.tile([128, 128], BF16, tag="Cb")
                nc.tensor.transpose(pst[:], Wm_sh[t][:], ident_bf[:])
                evac(WmT_sh[:, t * 128:(t + 1) * 128], pst[:], which="a")
            ag_wm_in = dramp.tile([128, BSH], BF16)
            ag_wm_out = dramp.tile([NCORES, 128, BSH], BF16, addr_space="Shared")
            nc.sync.dma_start(out=ag_wm_in[:], in_=WmT_sh[:])
            nc.gpsimd.collective_compute(
                "AllGather", ALU.bypass,
                replica_groups=[list(range(NCORES))],
                ins=[ag_wm_in.opt()], outs=[ag_wm_out.opt()])

            # =========== Phase 3b: inter trees + AllGather #2 (inter) ===========
            for t in range(2):
                for h in range(2):
                    inter_tree(t, h)
            for t in range(2):
                pst2 = psC.tile([128, 128], BF16, tag="Cb")
                nc.tensor.transpose(pst2[:], inter_sh[t][:], ident_bf[:])
                evac(intT_sh[:, t * 128:(t + 1) * 128], pst2[:], which="a")
            ag_int_in = dramp.tile([128, BSH], BF16)
            ag_int_out = dramp.tile([NCORES, 128, BSH], BF16, addr_space="Shared")
            nc.sync.dma_start(out=ag_int_in[:], in_=intT_sh[:])
            nc.gpsimd.collective_compute(
                "AllGather", ALU.bypass,
                replica_groups=[list(range(NCORES))],
                ins=[ag_int_in.opt()], outs=[ag_int_out.opt()])

            tail_group(3)
            tail_group(4)

            # BN1 on gathered WmT (full B)
            nc.sync.dma_start(
                out=WmT_all[:], in_=ag_wm_out[:].rearrange("r d b -> d r b"))
            bn1g_s = load_const(bn1g, [128, 1], "c13")
            bn1b_s = load_const(bn1b, [128, 1], "c14")
            st1 = smallp.tile([128, 4, 6], F32, tag="st1")
            for i in range(4):
                nc.vector.bn_stats(st1[:, i, :], WmT_all[:, i * 512:(i + 1) * 512])
            bn1_scale, bn1_shift = bn_finalize(st1[:], bn1g_s, bn1b_s)
            nc.vector.tensor_scalar(WmT_nb[:], WmT_all[:], bn1_scale[:, 0:1],
                                    bn1_shift[:, 0:1], op0=ALU.mult, op1=ALU.add)

            # deferred topk + transpose into tamT (DVE; concurrent with the
            # ACT-routed tucker evacuations below)
            for g, nt, raw in ta_raw:
                tam_nc = smallp.tile([128, 128], BF16, tag="tamnc")
                topk_mask_mul(raw[:], tam_nc[:])
                ptT = psC.tile([128, 128], BF16, tag="Cb")
                nc.tensor.transpose(ptT[:], tam_nc[:], ident_bf[:])
                evac(tamT_s[:, g * 512 + nt * 128:g * 512 + (nt + 1) * 128],
                     ptT[:], which="v")
            psC.release()

            # =========== Phase 4: scores ===========
            psD = tc.alloc_tile_pool(name="psD", bufs=1, space="PSUM")
            NLAST = NSH - 4 * 512               # 452 valid cols in group 4
            quad_i = [0]

            def quad_tile():
                quad_i[0] += 1
                pool = psD if quad_i[0] % 3 == 0 else psA
                return pool.tile([128, 2, 512], F32, tag="D" if pool is psD else "A",
                                 name=f"psq{quad_i[0]}")

            def score_branch(lhsT, outd, rhs, ev):
                for bt in range(NB_FULL):
                    r0 = bt * 128
                    for half in range(2):
                        psq = quad_tile()
                        for gg in range(2):
                            g = half * 2 + gg
                            mm(psq[:, gg, :], lhsT[:, r0:r0 + 128],
                               rhs[:, g * 512:(g + 1) * 512])
                        stq = stagep.tile([128, 1024], BF16, tag="stq")
                        eng = evac(stq[:], psq[:].rearrange("p g n -> p (g n)"),
                                   which=ev)
                        (nc.sync if eng == "v" else nc.scalar).dma_start(
                            out=outd[r0:r0 + 128, half * 1024:(half + 1) * 1024],
                            in_=stq[:])
                    ps4 = psB.tile([128, 512], F32, tag="B")
                    mm(ps4[:], lhsT[:, r0:r0 + 128], rhs[:, 4 * 512:5 * 512])
                    st4 = stagep.tile([128, 512], BF16, tag="st4")
                    eng = evac(st4[:], ps4[:], which=ev)
                    (nc.sync if eng == "v" else nc.scalar).dma_start(
                        out=outd[r0:r0 + 128, 2048:NSH], in_=st4[:, 0:NLAST])

            # tucker first: evacs on ACT only, so DVE can run the deferred topk
            score_branch(WmT_nb, tucker, tsT_s, ev="a")

            nc.sync.dma_start(
                out=intT_all[:], in_=ag_int_out[:].rearrange("r d b -> d r b"))
            score_branch(intT_all, poss, tamT_s, ev=None)
            psD.release()

    nc.finalize()
    return nc


# ---------------------------------------------------------------------------
# host side
# ---------------------------------------------------------------------------

BF = np.dtype(ml_dtypes.bfloat16)


def _to_np(x, dt=np.float32):
    return np.ascontiguousarray(np.asarray(x), dtype=dt)


def prepare_in_maps(inputs):
    head = _to_np(inputs["head_vector"])        # [B, E]
    rel = _to_np(inputs["relation_vector"])     # [B, E]
    ridx = np.ascontiguousarray(np.asarray(inputs["relation_index"]).astype(np.int32))
    tailv = _to_np(inputs["tail_vector"])       # [N, E]
    codebook = _to_np(inputs["codebook"])       # [R2, C, C]
    core = _to_np(inputs["core"])               # [C, C, C]

    cbT_host = np.ascontiguousarray(
        codebook.transpose(0, 2, 1).reshape(2 * R2, C * C // 2)).astype(BF)
    core_dc_host = np.ascontiguousarray(
        core.transpose(0, 2, 1).reshape(C, C * C)).astype(BF)

    headT_full = np.ascontiguousarray(head.T).astype(BF)   # [E, B]
    relT_full = np.ascontiguousarray(rel.T).astype(BF)     # [E, B]
    tailT_full = np.ascontiguousarray(tailv.T).astype(BF)  # [E, N]

    def chunked_bias(k, nk):
        return np.ascontiguousarray(_to_np(inputs[k]).reshape(nk, 128).T)

    col = lambda k: _to_np(inputs[k]).reshape(128, 1)
    wcast = lambda k: _to_np(inputs[k]).astype(BF)
    weights_common = {
        "hsw1": wcast("hsw1"), "hsb1": chunked_bias("hsb1", 4),
        "hsw2": wcast("hsw2"), "hsb2": col("hsb2"),
        "rsw1": wcast("rsw1"), "rsb1": chunked_bias("rsb1", 4),
        "rsw2": wcast("rsw2"), "rsb2": col("rsb2"),
        "tsw1": wcast("tsw1"), "tsb1": chunked_bias("tsb1", 4),
        "tsw2": wcast("tsw2"), "tsb2": col("tsb2"),
        "taw1": wcast("taw1"), "tab1": chunked_bias("tab1", 4),
        "taw2": wcast("taw2"),
        "tab2": _to_np(inputs["tab2"]).reshape(1, C).astype(BF),
        "hrw1": wcast("hrw1"), "hrb1": chunked_bias("hrb1", 2),
        "hrw2": wcast("hrw2"), "hrb2": chunked_bias("hrb2", 2),
        "hrw3": wcast("hrw3"), "hrb3": col("hrb3"),
        "bn0g": col("bn0_g"), "bn0b": col("bn0_b"),
        "bn1g": col("bn1_g"), "bn1b": col("bn1_b"),
        "cbT": cbT_host, "core_dc": core_dc_host,
    }

    in_maps = []
    for k in range(NCORES):
        b0 = k * BSH
        n0 = k * NSH
        # rotate headT so THIS core's 256 b-columns come first; BN0 stats are
        # order-invariant and slices [0:256] are "my" shard on every core.
        headT_k = np.ascontiguousarray(np.roll(headT_full, -b0, axis=1))
        tailT_k = np.zeros((E, NPAD), BF)
        tailT_k[:, :NSH] = tailT_full[:, n0:n0 + NSH]
        m = dict(weights_common)
        m["headT"] = headT_k
        m["relT"] = np.ascontiguousarray(relT_full[:, b0:b0 + BSH])
        m["tailT"] = tailT_k
        ri = ridx[b0:b0 + BSH]
        m["relidx"] = np.ascontiguousarray(
            np.stack([2 * ri, 2 * ri + 1], axis=1))
        in_maps.append(m)
    return in_maps


def assemble_outputs(results):
    tuckers, posses = [], []
    for k in range(NCORES):
        r = results[k]
        tuckers.append(np.asarray(r["tucker"]).astype(np.float32))
        posses.append(np.asarray(r["poss"]).astype(np.float32))
    tucker_full = np.concatenate(tuckers, axis=1)
    poss_full = np.concatenate(posses, axis=1)
    return tucker_full, poss_full


def kernel(**inputs):
    if "prog" not in _PROG_CACHE:
        _PROG_CACHE["prog"] = build_program()
    nc = _PROG_CACHE["prog"]
    in_maps = prepare_in_maps(inputs)
    res = run_bass_kernel_spmd(nc, in_maps, list(range(NCORES)))
    return assemble_outputs(res.results)
